# revision 11
# baseline (speedup 1.0000x reference)
"""RNN-T joint network kernel for Trainium2 (Bass/Tile), 8-core data-parallel.

Problem: out[b,t,u,:] = tanh(enc[b,t]@W_enc + b_enc + dec[b,u]@W_dec + b_dec) @ W_out + b_out
Shapes: B=8, T=256, U=64, D=512, J=640, V=1024 (all fp32).

Sharding: data-parallel over batch B across the 8 NeuronCores (1 batch element
per core). Per core the dominant work is the joint matmul (T,U,J)x(J,V):
655,360 PE streaming cycles @2.4GHz = 273us, vs ~187us HBM write for the 64MB
output slice -> PE-bound "ridge" regime. The main loop runs at the PE
streaming floor (216ns per 512-col bf16 matmul = 512cyc @2.4GHz + ~3cyc NX
issue @1.2GHz), so everything else is setup latency + drain tail.

Per-core plan (all J-major layouts so J is the matmul contraction partition dim):
  prewarm: ~10 dummy matmuls on a memset tile keep the PE busy from engine boot
          so the HAM clock gate un-throttles (1.2->2.4GHz) before the real work.
  setup:  input loads are HBM-bandwidth-shared across queues (~290GB/s/core
          aggregate), so the projection operands (critical path) are packed
          host-side into two partition-major bf16 "mega" tensors with
          multi-KB lines, both loaded enc-first on the sync HWDGE queue
          (it starts ~2.5us before the scalar queue). W_out follows in
          first-needed-first order split across the sync tail and the scalar
          queue. b_out is loaded as a single 4KB row and replicated to 128
          partitions on-chip via gpsimd.partition_broadcast (instead of a
          0.5MB broadcast-read DMA that would steal phase-1 bandwidth).
          Projections run off slices of the mega tiles (enc side first, so
          the PE overlaps the dec-side load): enc_projT[j,t] = W_enc^T@encT;
          dec_projT[j,u] = W_dec^T @ decT + (b_enc+b_dec). Both PSUM drains
          happen on the otherwise-idle DVE (enc: copy, dec: per-partition
          tensor_scalar_add of the bias) so the in-order ACT engine reaches
          the first tanh batch immediately. Small filler matmuls ride out
          the W_out-load wait so the HAM window never re-throttles.
  main:   for each u: hT[j,t] = tanh(enc_projT[j,t] + dec_projT[j,u]) via ACT
          (bias = per-partition dec column, broadcast along free dim t);
          joint matmul out(t,1024) = hT^T @ W_out in bf16 (jc outer, vv inner
          so the stationary operand is reused across the two 512-col psum
          halves); DVE adds broadcast b_out while draining PSUM->SBUF; stores
          go per (tt,uu) in 512KB chunks, even uu on the sync queue, odd uu
          on the scalar queue. Scalar-queue stores are deferred by one
          u-block: the ACT engine is in-order, so an enqueue waiting on a
          fresh DVE drain must not sit in front of the next u-block's tanh
          batch (that would stall the PE). Stores are bf16 (upconverted to
          f32 on the host; ~0.1% extra error vs the 2e-2 budget) to halve
          store bandwidth. The final u-block drains in 256-col quarters with
          stores fanned across both queues to shorten the tail.
"""

import numpy as np
from contextlib import ExitStack

from concourse import bacc, bass, tile
from concourse.bass import mybir
from concourse.bass_utils import run_bass_kernel_spmd

F32 = mybir.dt.float32
BF16 = mybir.dt.bfloat16
ACT_F = mybir.ActivationFunctionType

B, T, U = 8, 256, 64
D, J, V = 512, 640, 1024
NJC = J // 128   # 5 contraction chunks of the joint matmul
NDC = D // 128   # 4 contraction chunks of the projections
UB = 4           # u-block whose tanh tiles are staged together
NVB = V // 512   # 2 psum banks per joint output tile

# mega layouts (bf16 cols per partition line), loaded enc-first so the enc
# projections start while the dec side is still in flight:
#   mega_enc: [encT dc0..3 (4x256) | w_enc dc0..3 (4x640)] = 3584 cols
#   mega_dec: [decT dc0..3 (4x64)  | w_dec dc0..3 (4x640)] = 2816 cols
ME_COLS = NDC * T + NDC * J  # 3584
MD_COLS = NDC * U + NDC * J  # 2816


def build_program() -> bass.Bass:
    nc = bacc.Bacc("TRN2", target_bir_lowering=False, debug=False)

    mega_enc_d = nc.declare_dram_parameter("mega_enc", [128, ME_COLS], BF16, isOutput=False)
    mega_dec_d = nc.declare_dram_parameter("mega_dec", [128, MD_COLS], BF16, isOutput=False)
    bbT_d = nc.declare_dram_parameter("bbT", [128, NJC], F32, isOutput=False)  # (b_enc+b_dec)[jc*128+p]
    w_out = nc.declare_dram_parameter("w_out", [J, V], BF16, isOutput=False)
    b_out = nc.declare_dram_parameter("b_out", [V], F32, isOutput=False)
    out = nc.declare_dram_parameter("out", [T, U, V], BF16, isOutput=True)

    with tile.TileContext(nc) as tc, ExitStack() as ctx:
        const = ctx.enter_context(tc.tile_pool(name="const", bufs=1))

        # --- resident tiles ----------------------------------------------
        w_out_sb = [
            [const.tile([128, 512], BF16, tag=f"wout{jc}_{vv}", name=f"wout{jc}_{vv}") for vv in range(NVB)]
            for jc in range(NJC)
        ]
        bias_rep = const.tile([128, V], F32, tag="brep")
        b_out_row = const.tile([1, V], F32, tag="brow")
        bbt = const.tile([128, NJC], F32, tag="bbt")
        enc_projT = [const.tile([128, T], F32, tag=f"ep{jc}", name=f"ep{jc}") for jc in range(NJC)]
        dec_projT = [const.tile([128, U], F32, tag=f"dp{jc}", name=f"dp{jc}") for jc in range(NJC)]

        warm_w = const.tile([128, 128], BF16, tag="warmw")
        warm_x = const.tile([128, 512], BF16, tag="warmx")
        nc.vector.memset(warm_w[:], 0.0)
        nc.vector.memset(warm_x[:], 0.0)

        with (
            tc.tile_pool(name="setup", bufs=1) as setup_keep,
            tc.tile_pool(name="setup_ps", bufs=2, space="PSUM") as setup_ps,
            tc.tile_pool(name="warm_ps", bufs=1, space="PSUM") as warm_pool,
        ):
            # --- PE prewarm: dummy matmuls from engine boot until the input
            # loads land, so HAM lifts the clock gate before the projections.
            wps = warm_pool.tile([128, 512], F32, tag="wps")
            for _ in range(10):
                nc.tensor.matmul(wps[:], warm_w[:], warm_x[:], start=True, stop=True)

            # --- input loads ---------------------------------------------
            # both megas on the sync queue (it starts ~2.5us before the
            # scalar HWDGE queue), enc side first; W_out split across the
            # scalar queue and the sync-queue tail in first-needed order
            mega_e = setup_keep.tile([128, ME_COLS], BF16, tag="mega_e")
            mega_d = setup_keep.tile([128, MD_COLS], BF16, tag="mega_d")
            nc.sync.dma_start(out=mega_e[:], in_=mega_enc_d[:, :])
            nc.sync.dma_start(out=mega_d[:], in_=mega_dec_d[:, :])
            nc.scalar.dma_start(out=b_out_row[:], in_=b_out[:].unsqueeze(0))
            # W_out halves in first-needed order: the first psum tiles run
            # vv-outer, so all five vv=0 halves are wanted before any vv=1
            for vv in range(NVB):
                for jc in range(NJC):
                    eng = nc.scalar if (jc + vv) % 2 == 0 else nc.sync
                    eng.dma_start(
                        out=w_out_sb[jc][vv][:],
                        in_=w_out[jc * 128 : (jc + 1) * 128, vv * 512 : (vv + 1) * 512],
                    )
            nc.gpsimd.dma_start(out=bbt[:], in_=bbT_d[:, :])
            nc.gpsimd.partition_broadcast(bias_rep[:], b_out_row[:])

            # --- projections (bf16 operands, f32 PSUM, DVE drains) ----------
            # enc side first: it only needs mega_e, which lands well before
            # mega_d, so the PE overlaps the dec-side load
            for jc in range(NJC):
                ps = setup_ps.tile([128, T], F32, tag="proj")
                for dc in range(NDC):
                    nc.tensor.matmul(
                        ps[:], mega_e[:, NDC * T + dc * J + jc * 128 :][:, :128], mega_e[:, dc * T : (dc + 1) * T],
                        start=(dc == 0), stop=(dc == NDC - 1),
                    )
                nc.vector.tensor_scalar_add(enc_projT[jc][:], ps[:], 0.0)
            for jc in range(NJC):
                ps = setup_ps.tile([128, U], F32, tag="projd")
                for dc in range(NDC):
                    nc.tensor.matmul(
                        ps[:], mega_d[:, NDC * U + dc * J + jc * 128 :][:, :128], mega_d[:, dc * U : (dc + 1) * U],
                        start=(dc == 0), stop=(dc == NDC - 1),
                    )
                # fold b_enc+b_dec into dec_projT during the PSUM->SBUF drain
                nc.vector.tensor_scalar_add(dec_projT[jc][:], ps[:], bbt[:, jc : jc + 1])

            # small fillers ride out the W_out-load wait so the HAM activity
            # window never sees the PE idle and the main loop starts at 2.4GHz
            for _ in range(20):
                nc.tensor.matmul(wps[:, :64], warm_w[:], warm_x[:, :64], start=True, stop=True)

        # --- main loop over u-blocks --------------------------------------
        h_pool = ctx.enter_context(tc.tile_pool(name="h", bufs=4))
        ub0_pool = ctx.enter_context(tc.tile_pool(name="ub0", bufs=1))
        st_pool = ctx.enter_context(tc.tile_pool(name="stage", bufs=6))
        mm_ps = ctx.enter_context(tc.tile_pool(name="mm_ps", bufs=4, space="PSUM"))

        deferred = []  # scalar-queue stores held back one u-block (see docstring)
        NUB = U // UB
        for ub in range(NUB):
            u0 = ub * UB
            hT = [
                [h_pool.tile([128, T], BF16, tag=f"h{jc}_{uu}", name=f"h{jc}_{uu}") for uu in range(UB)]
                for jc in range(NJC)
            ]
            wide = {}
            if ub == 0:
                # first two u values: add the dec bias on the idle DVE and run
                # ONE wide 1280-col tanh per u (1.36us) instead of five biased
                # 256-col tanh (2.55us serial) -- the ACT chain gates main start
                for uu in (0, 1):
                    joint = ub0_pool.tile([128, NJC * T], F32, tag=f"j{uu}", name=f"j{uu}")
                    for jc in range(NJC):
                        nc.vector.tensor_scalar_add(
                            joint[:, jc * T : (jc + 1) * T],
                            enc_projT[jc][:],
                            dec_projT[jc][:, u0 + uu : u0 + uu + 1],
                        )
                    w = ub0_pool.tile([128, NJC * T], BF16, tag=f"hw{uu}", name=f"hw{uu}")
                    nc.scalar.activation(w[:], joint[:], ACT_F.Tanh)
                    wide[uu] = w
            # uu-major so the first joint matmul's operands are ready earliest
            for uu in range(UB):
                if uu in wide:
                    continue
                for jc in range(NJC):
                    nc.scalar.activation(
                        hT[jc][uu][:],
                        enc_projT[jc][:],
                        ACT_F.Tanh,
                        bias=dec_projT[jc][:, u0 + uu : u0 + uu + 1],
                        scale=1.0,
                    )

            def h_ap(jc, uu, tt, _wide=wide, _hT=hT):
                if uu in _wide:
                    return _wide[uu][:, jc * T + tt * 128 : jc * T + (tt + 1) * 128]
                return _hT[jc][uu][:, tt * 128 : (tt + 1) * 128]
            # flush the previous u-block's scalar-queue stores now that this
            # u-block's tanh batch is already queued ahead of them on ACT
            for stg, tt_, u_ in deferred:
                nc.scalar.dma_start(out=out[tt_ * 128 : (tt_ + 1) * 128, u_, :], in_=stg[:])
            deferred = []
            last_ub = ub == NUB - 1
            for tt in range(T // 128):
                for uu in range(UB):
                    ps = mm_ps.tile([128, V], F32, tag="mm")
                    # jc outer / vv inner: the stationary operand (the h tile)
                    # is reused across both 512-col psum halves. The first tt
                    # group runs vv outer instead, so it can start on the
                    # vv=0 W_out halves while the vv=1 halves are in flight.
                    loop = (
                        [(jc, vv) for vv in range(NVB) for jc in range(NJC)]
                        if ub == 0 and tt == 0
                        else [(jc, vv) for jc in range(NJC) for vv in range(NVB)]
                    )
                    for jc, vv in loop:
                        if True:
                            nc.tensor.matmul(
                                ps[:, vv * 512 : (vv + 1) * 512],
                                h_ap(jc, uu, tt),
                                w_out_sb[jc][vv][:],
                                start=(jc == 0),
                                stop=(jc == NJC - 1),
                            )
                    if last_ub:
                        # drain in quarters, stores fanned across both queues,
                        # so the tail after the final matmul is as short as
                        # possible
                        for qt in range(4):
                            stg = st_pool.tile([128, 256], BF16, tag=f"stq{qt}", name=f"stq{qt}")
                            nc.vector.tensor_add(
                                stg[:], ps[:, qt * 256 : (qt + 1) * 256],
                                bias_rep[:, qt * 256 : (qt + 1) * 256],
                            )
                            eng = nc.sync if qt % 2 == 0 else nc.scalar
                            eng.dma_start(
                                out=out[tt * 128 : (tt + 1) * 128, u0 + uu, qt * 256 : (qt + 1) * 256],
                                in_=stg[:],
                            )
                        continue
                    # drain PSUM -> SBUF while adding the broadcast b_out
                    stage = st_pool.tile([128, V], BF16, tag="st_e" if uu % 2 == 0 else "st_o")
                    nc.vector.tensor_add(stage[:], ps[:], bias_rep[:])
                    if uu % 2 == 0:
                        nc.sync.dma_start(
                            out=out[tt * 128 : (tt + 1) * 128, u0 + uu, :],
                            in_=stage[:],
                        )
                    else:
                        deferred.append((stage, tt, u0 + uu))

    nc.finalize()
    return nc


_PROGRAM = None


def make_in_maps(enc_out, dec_out, W_enc, b_enc, W_dec, b_dec, W_out, b_out):
    import ml_dtypes

    bf16 = ml_dtypes.bfloat16
    bb = np.asarray(b_enc, np.float32) + np.asarray(b_dec, np.float32)
    bbT = np.ascontiguousarray(bb.reshape(NJC, 128).T)
    w_enc_m = (
        np.asarray(W_enc, np.float32).astype(bf16).reshape(NDC, 128, J).transpose(1, 0, 2).reshape(128, NDC * J)
    )
    w_dec_m = (
        np.asarray(W_dec, np.float32).astype(bf16).reshape(NDC, 128, J).transpose(1, 0, 2).reshape(128, NDC * J)
    )
    w_out_bf = np.asarray(W_out, np.float32).astype(bf16)
    b_out_f = np.asarray(b_out, np.float32)
    enc = np.asarray(enc_out, np.float32)
    dec = np.asarray(dec_out, np.float32)
    in_maps = []
    for b in range(B):
        encT = enc[b, :, 0, :].T.astype(bf16)  # [D, T]
        decT = dec[b, 0, :, :].T.astype(bf16)  # [D, U]
        enc_m = encT.reshape(NDC, 128, T).transpose(1, 0, 2).reshape(128, NDC * T)
        dec_m = decT.reshape(NDC, 128, U).transpose(1, 0, 2).reshape(128, NDC * U)
        in_maps.append(
            {
                "mega_enc": np.ascontiguousarray(np.concatenate([enc_m, w_enc_m], axis=1)),
                "mega_dec": np.ascontiguousarray(np.concatenate([dec_m, w_dec_m], axis=1)),
                "bbT": bbT,
                "w_out": w_out_bf,
                "b_out": b_out_f,
            }
        )
    return in_maps


def kernel(enc_out, dec_out, W_enc, b_enc, W_dec, b_dec, W_out, b_out):
    global _PROGRAM
    if _PROGRAM is None:
        _PROGRAM = build_program()

    in_maps = make_in_maps(enc_out, dec_out, W_enc, b_enc, W_dec, b_dec, W_out, b_out)
    res = run_bass_kernel_spmd(_PROGRAM, in_maps, list(range(B)))
    return np.stack([res.results[b]["out"].astype(np.float32) for b in range(B)], axis=0)


# revision 12
# speedup vs baseline: 1.0107x; 1.0107x over previous
"""RNN-T joint network kernel for Trainium2 (Bass/Tile), 8-core data-parallel.

Problem: out[b,t,u,:] = tanh(enc[b,t]@W_enc + b_enc + dec[b,u]@W_dec + b_dec) @ W_out + b_out
Shapes: B=8, T=256, U=64, D=512, J=640, V=1024 (all fp32).

Sharding: data-parallel over batch B across the 8 NeuronCores (1 batch element
per core). Per core the dominant work is the joint matmul (T,U,J)x(J,V):
655,360 PE streaming cycles @2.4GHz = 273us, vs ~187us HBM write for the 64MB
output slice -> PE-bound "ridge" regime. The main loop runs at the PE
streaming floor (216ns per 512-col bf16 matmul = 512cyc @2.4GHz + ~3cyc NX
issue @1.2GHz), so everything else is setup latency + drain tail.

Per-core plan (all J-major layouts so J is the matmul contraction partition dim):
  prewarm: ~10 dummy matmuls on a memset tile keep the PE busy from engine boot
          so the HAM clock gate un-throttles (1.2->2.4GHz) before the real work.
  setup:  input loads are HBM-bandwidth-shared across queues (~290GB/s/core
          aggregate), so the projection operands (critical path) are packed
          host-side into two partition-major bf16 "mega" tensors with
          multi-KB lines, both loaded enc-first on the sync HWDGE queue
          (it starts ~2.5us before the scalar queue). W_out follows in
          first-needed-first order split across the sync tail and the scalar
          queue. b_out is loaded as a single 4KB row and replicated to 128
          partitions on-chip via gpsimd.partition_broadcast (instead of a
          0.5MB broadcast-read DMA that would steal phase-1 bandwidth).
          Projections run off slices of the mega tiles (enc side first, so
          the PE overlaps the dec-side load): enc_projT[j,t] = W_enc^T@encT;
          dec_projT[j,u] = W_dec^T @ decT + (b_enc+b_dec). Both PSUM drains
          happen on the otherwise-idle DVE (enc: copy, dec: per-partition
          tensor_scalar_add of the bias) so the in-order ACT engine reaches
          the first tanh batch immediately. Small filler matmuls ride out
          the W_out-load wait so the HAM window never re-throttles.
  main:   for each u: hT[j,t] = tanh(enc_projT[j,t] + dec_projT[j,u]) via ACT
          (bias = per-partition dec column, broadcast along free dim t);
          joint matmul out(t,1024) = hT^T @ W_out in bf16 (jc outer, vv inner
          so the stationary operand is reused across the two 512-col psum
          halves); DVE adds broadcast b_out while draining PSUM->SBUF; stores
          go per (tt,uu) in 512KB chunks, even uu on the sync queue, odd uu
          on the scalar queue. Scalar-queue stores are deferred by one
          u-block: the ACT engine is in-order, so an enqueue waiting on a
          fresh DVE drain must not sit in front of the next u-block's tanh
          batch (that would stall the PE). Stores are bf16 (upconverted to
          f32 on the host; ~0.1% extra error vs the 2e-2 budget) to halve
          store bandwidth. The final u-block drains in 256-col quarters with
          stores fanned across both queues to shorten the tail.
"""

import numpy as np
from contextlib import ExitStack

from concourse import bacc, bass, tile
from concourse.bass import mybir
from concourse.bass_utils import run_bass_kernel_spmd

F32 = mybir.dt.float32
BF16 = mybir.dt.bfloat16
ACT_F = mybir.ActivationFunctionType

B, T, U = 8, 256, 64
D, J, V = 512, 640, 1024
NJC = J // 128   # 5 contraction chunks of the joint matmul
NDC = D // 128   # 4 contraction chunks of the projections
UB = 4           # u-block whose tanh tiles are staged together
NVB = V // 512   # 2 psum banks per joint output tile

# mega layouts (bf16 cols per partition line), loaded enc-first so the enc
# projections start while the dec side is still in flight:
#   mega_enc: [encT dc0..3 (4x256) | w_enc dc0..3 (4x640)] = 3584 cols
#   mega_dec: [decT dc0..3 (4x64)  | w_dec dc0..3 (4x640)] = 2816 cols
ME_COLS = NDC * T + NDC * J  # 3584
MD_COLS = NDC * U + NDC * J  # 2816


def build_program() -> bass.Bass:
    nc = bacc.Bacc("TRN2", target_bir_lowering=False, debug=False)

    mega_enc_d = nc.declare_dram_parameter("mega_enc", [128, ME_COLS], BF16, isOutput=False)
    mega_dec_d = nc.declare_dram_parameter("mega_dec", [128, MD_COLS], BF16, isOutput=False)
    bbT_d = nc.declare_dram_parameter("bbT", [128, NJC], F32, isOutput=False)  # (b_enc+b_dec)[jc*128+p]
    w_out = nc.declare_dram_parameter("w_out", [J, V], BF16, isOutput=False)
    b_out = nc.declare_dram_parameter("b_out", [V], F32, isOutput=False)
    out = nc.declare_dram_parameter("out", [T, U, V], BF16, isOutput=True)

    with tile.TileContext(nc) as tc, ExitStack() as ctx:
        const = ctx.enter_context(tc.tile_pool(name="const", bufs=1))

        # --- resident tiles ----------------------------------------------
        w_out_sb = [const.tile([128, V], BF16, tag=f"wout{jc}", name=f"wout{jc}") for jc in range(NJC)]
        bias_rep = const.tile([128, V], F32, tag="brep")
        b_out_row = const.tile([1, V], F32, tag="brow")
        bbt = const.tile([128, NJC], F32, tag="bbt")
        enc_projT = [const.tile([128, T], F32, tag=f"ep{jc}", name=f"ep{jc}") for jc in range(NJC)]
        dec_projT = [const.tile([128, U], F32, tag=f"dp{jc}", name=f"dp{jc}") for jc in range(NJC)]

        warm_w = const.tile([128, 128], BF16, tag="warmw")
        warm_x = const.tile([128, 512], BF16, tag="warmx")
        nc.vector.memset(warm_w[:], 0.0)
        nc.vector.memset(warm_x[:], 0.0)

        with (
            tc.tile_pool(name="setup", bufs=1) as setup_keep,
            tc.tile_pool(name="setup_ps", bufs=2, space="PSUM") as setup_ps,
            tc.tile_pool(name="warm_ps", bufs=1, space="PSUM") as warm_pool,
        ):
            # --- PE prewarm: dummy matmuls from engine boot until the input
            # loads land, so HAM lifts the clock gate before the projections.
            wps = warm_pool.tile([128, 512], F32, tag="wps")
            for _ in range(10):
                nc.tensor.matmul(wps[:], warm_w[:], warm_x[:], start=True, stop=True)

            # --- input loads ---------------------------------------------
            # both megas on the sync queue (it starts ~2.5us before the
            # scalar HWDGE queue), enc side first; W_out split across the
            # scalar queue and the sync-queue tail in first-needed order
            mega_e = setup_keep.tile([128, ME_COLS], BF16, tag="mega_e")
            mega_d = setup_keep.tile([128, MD_COLS], BF16, tag="mega_d")
            nc.sync.dma_start(out=mega_e[:], in_=mega_enc_d[:, :])
            nc.sync.dma_start(out=mega_d[:], in_=mega_dec_d[:, :])
            nc.scalar.dma_start(out=b_out_row[:], in_=b_out[:].unsqueeze(0))
            nc.scalar.dma_start(out=w_out_sb[0][:], in_=w_out[0:128, :])
            nc.scalar.dma_start(out=w_out_sb[1][:], in_=w_out[128:256, :])
            nc.scalar.dma_start(out=w_out_sb[2][:], in_=w_out[256:384, :])
            nc.sync.dma_start(out=w_out_sb[3][:], in_=w_out[384:512, :])
            nc.sync.dma_start(out=w_out_sb[4][:], in_=w_out[512:640, :])
            nc.gpsimd.dma_start(out=bbt[:], in_=bbT_d[:, :])
            nc.gpsimd.partition_broadcast(bias_rep[:], b_out_row[:])

            # --- projections (bf16 operands, f32 PSUM, DVE drains) ----------
            # enc side first: it only needs mega_e, which lands well before
            # mega_d, so the PE overlaps the dec-side load
            for jc in range(NJC):
                ps = setup_ps.tile([128, T], F32, tag="proj")
                for dc in range(NDC):
                    nc.tensor.matmul(
                        ps[:], mega_e[:, NDC * T + dc * J + jc * 128 :][:, :128], mega_e[:, dc * T : (dc + 1) * T],
                        start=(dc == 0), stop=(dc == NDC - 1),
                    )
                nc.vector.tensor_scalar_add(enc_projT[jc][:], ps[:], 0.0)
            for jc in range(NJC):
                ps = setup_ps.tile([128, U], F32, tag="projd")
                for dc in range(NDC):
                    nc.tensor.matmul(
                        ps[:], mega_d[:, NDC * U + dc * J + jc * 128 :][:, :128], mega_d[:, dc * U : (dc + 1) * U],
                        start=(dc == 0), stop=(dc == NDC - 1),
                    )
                # fold b_enc+b_dec into dec_projT during the PSUM->SBUF drain
                nc.vector.tensor_scalar_add(dec_projT[jc][:], ps[:], bbt[:, jc : jc + 1])

            # small fillers ride out the W_out-load wait so the HAM activity
            # window never sees the PE idle and the main loop starts at 2.4GHz
            for _ in range(20):
                nc.tensor.matmul(wps[:, :64], warm_w[:], warm_x[:, :64], start=True, stop=True)

        # --- main loop over u-blocks --------------------------------------
        h_pool = ctx.enter_context(tc.tile_pool(name="h", bufs=4))
        st_pool = ctx.enter_context(tc.tile_pool(name="stage", bufs=6))
        mm_ps = ctx.enter_context(tc.tile_pool(name="mm_ps", bufs=4, space="PSUM"))

        deferred = []  # scalar-queue stores held back one u-block (see docstring)
        NUB = U // UB
        for ub in range(NUB):
            u0 = ub * UB
            hT = [
                [h_pool.tile([128, T], BF16, tag=f"h{jc}_{uu}", name=f"h{jc}_{uu}") for uu in range(UB)]
                for jc in range(NJC)
            ]
            # uu-major so the first joint matmul's operands are ready earliest
            for uu in range(UB):
                for jc in range(NJC):
                    nc.scalar.activation(
                        hT[jc][uu][:],
                        enc_projT[jc][:],
                        ACT_F.Tanh,
                        bias=dec_projT[jc][:, u0 + uu : u0 + uu + 1],
                        scale=1.0,
                    )
            # flush the previous u-block's scalar-queue stores now that this
            # u-block's tanh batch is already queued ahead of them on ACT
            for stg, tt_, u_ in deferred:
                nc.scalar.dma_start(out=out[tt_ * 128 : (tt_ + 1) * 128, u_, :], in_=stg[:])
            deferred = []
            last_ub = ub == NUB - 1
            for tt in range(T // 128):
                for uu in range(UB):
                    ps = mm_ps.tile([128, V], F32, tag="mm")
                    # jc outer / vv inner: the stationary operand (the h tile)
                    # is reused across both 512-col psum halves
                    for jc in range(NJC):
                        for vv in range(NVB):
                            nc.tensor.matmul(
                                ps[:, vv * 512 : (vv + 1) * 512],
                                hT[jc][uu][:, tt * 128 : (tt + 1) * 128],
                                w_out_sb[jc][:, vv * 512 : (vv + 1) * 512],
                                start=(jc == 0),
                                stop=(jc == NJC - 1),
                            )
                    if last_ub:
                        # drain in quarters, stores fanned across both queues,
                        # so the tail after the final matmul is as short as
                        # possible
                        for qt in range(4):
                            stg = st_pool.tile([128, 256], BF16, tag=f"stq{qt}", name=f"stq{qt}")
                            nc.vector.tensor_add(
                                stg[:], ps[:, qt * 256 : (qt + 1) * 256],
                                bias_rep[:, qt * 256 : (qt + 1) * 256],
                            )
                            eng = nc.sync if qt % 2 == 0 else nc.scalar
                            eng.dma_start(
                                out=out[tt * 128 : (tt + 1) * 128, u0 + uu, qt * 256 : (qt + 1) * 256],
                                in_=stg[:],
                            )
                        continue
                    # drain PSUM -> SBUF while adding the broadcast b_out
                    stage = st_pool.tile([128, V], BF16, tag="st_e" if uu % 2 == 0 else "st_o")
                    nc.vector.tensor_add(stage[:], ps[:], bias_rep[:])
                    if uu % 2 == 0:
                        nc.sync.dma_start(
                            out=out[tt * 128 : (tt + 1) * 128, u0 + uu, :],
                            in_=stage[:],
                        )
                    else:
                        deferred.append((stage, tt, u0 + uu))

    nc.finalize()
    return nc


_PROGRAM = None


def make_in_maps(enc_out, dec_out, W_enc, b_enc, W_dec, b_dec, W_out, b_out):
    import ml_dtypes

    bf16 = ml_dtypes.bfloat16
    bb = np.asarray(b_enc, np.float32) + np.asarray(b_dec, np.float32)
    bbT = np.ascontiguousarray(bb.reshape(NJC, 128).T)
    w_enc_m = (
        np.asarray(W_enc, np.float32).astype(bf16).reshape(NDC, 128, J).transpose(1, 0, 2).reshape(128, NDC * J)
    )
    w_dec_m = (
        np.asarray(W_dec, np.float32).astype(bf16).reshape(NDC, 128, J).transpose(1, 0, 2).reshape(128, NDC * J)
    )
    w_out_bf = np.asarray(W_out, np.float32).astype(bf16)
    b_out_f = np.asarray(b_out, np.float32)
    enc = np.asarray(enc_out, np.float32)
    dec = np.asarray(dec_out, np.float32)
    in_maps = []
    for b in range(B):
        encT = enc[b, :, 0, :].T.astype(bf16)  # [D, T]
        decT = dec[b, 0, :, :].T.astype(bf16)  # [D, U]
        enc_m = encT.reshape(NDC, 128, T).transpose(1, 0, 2).reshape(128, NDC * T)
        dec_m = decT.reshape(NDC, 128, U).transpose(1, 0, 2).reshape(128, NDC * U)
        in_maps.append(
            {
                "mega_enc": np.ascontiguousarray(np.concatenate([enc_m, w_enc_m], axis=1)),
                "mega_dec": np.ascontiguousarray(np.concatenate([dec_m, w_dec_m], axis=1)),
                "bbT": bbT,
                "w_out": w_out_bf,
                "b_out": b_out_f,
            }
        )
    return in_maps


def kernel(enc_out, dec_out, W_enc, b_enc, W_dec, b_dec, W_out, b_out):
    global _PROGRAM
    if _PROGRAM is None:
        _PROGRAM = build_program()

    in_maps = make_in_maps(enc_out, dec_out, W_enc, b_enc, W_dec, b_dec, W_out, b_out)
    res = run_bass_kernel_spmd(_PROGRAM, in_maps, list(range(B)))
    return np.stack([res.results[b]["out"].astype(np.float32) for b in range(B)], axis=0)
